# revision 36
# baseline (speedup 1.0000x reference)
"""Trainium2 Bass kernel for nn_Attention (B=8, N=2048, H=512).

Reference computation (per batch b):
    out   = lstm_out @ W^T + b          # [N, H]
    score = out @ out^T                 # [N, N]
    attn  = softmax(score, axis=-1)
    ctx   = attn @ lstm_out             # [N, H]

Sharding: data-parallel over batch B across the 8 NeuronCores (one batch
element per core); W/b replicated. Each core runs an identical single-core
NEFF (SPMD, no collectives).

Per-core algorithm (matmuls in bf16 with fp32 PSUM accumulation; softmax
stats in fp32 with the exact row-max subtraction, like the reference):
  1. x [2048, 512] f32 -> SBUF, cast to bf16 (x_bf, token-major).
  2. xT (h-major) via xbar DMA transpose (4 chunked SBUF->SBUF transposes).
  3. outT[h, n] = W @ xT + b (h-major); lhsT = W^T built via PE
     identity-matmul transposes (W is tiny).
  4. per 128-query block (1-deep software pipeline so PE never stalls):
     stage A: S = outT^T @ outT -> PSUM [128, 2048] fp32 (16 matmuls),
       with per-bank partial row-max on DVE trailing the matmuls;
       p = exp(S - m) -> bf16 (ACT, fused per-partition bias and
       accumulated row-sums); subtract I from p's diagonal block;
       pT via one xbar DMA transpose -> [128, 16, 128].
     stage B (previous block): ctx = pT^T @ x_bf + x_f32, scaled by
       1/rowsum ("residual" form keeps the dominant diagonal term in
       exact fp32: attn@x = ((p - I) @ x + x) / rowsum(p)); DMA out.
"""

import sys

sys.path.insert(0, "/opt/trn_rl_repo")

import numpy as np

import concourse.bass as bass
import concourse.tile as tile
from concourse import bacc, mybir
from concourse.bass_utils import run_bass_kernel_spmd
from concourse.masks import make_identity

B, N, H = 8, 2048, 512
P = 128          # partitions
NT = N // P      # 16 token tiles
HC = H // P      # 4 h-chunks
FT = N // 512    # 4 free-dim tiles of 512 over tokens

F32 = mybir.dt.float32
BF16 = mybir.dt.bfloat16
FP8 = mybir.dt.float8e4

_NC_CACHE = None


def _build(ctx, tc):
    nc = tc.nc
    x = nc.dram_tensor("x", [N, H], F32, kind="ExternalInput").ap()
    w = nc.dram_tensor("w", [H, H], F32, kind="ExternalInput").ap()
    bvec = nc.dram_tensor("bvec", [H], F32, kind="ExternalInput").ap()
    out = nc.dram_tensor("out", [N, H], F32, kind="ExternalOutput").ap()

    const = ctx.enter_context(tc.tile_pool(name="const", bufs=1))
    big = ctx.enter_context(tc.tile_pool(name="big", bufs=1))
    p_pool = ctx.enter_context(tc.tile_pool(name="p", bufs=3))
    pt_pool = ctx.enter_context(tc.tile_pool(name="pt", bufs=3))
    stats = ctx.enter_context(tc.tile_pool(name="stats", bufs=12))
    ctx_pool = ctx.enter_context(tc.tile_pool(name="ctxp", bufs=3))

    ps_mm = ctx.enter_context(tc.tile_pool(name="ps_mm", bufs=2, space="PSUM"))
    ps_stage = ctx.enter_context(tc.tile_pool(name="ps_stage", bufs=2, space="PSUM"))

    # --- constants ---
    ident = const.tile([P, P], BF16)
    make_identity(nc, ident[:])
    b_sb = const.tile([P, HC], F32)
    nc.gpsimd.dma_start(b_sb[:], bvec.rearrange("(c p) -> p c", p=P))
    # (W also goes over the gpsimd DMA queue so it doesn't serialize behind
    # the x-tile loads on the sync queue)

    # --- persistent big tensors (split into per-group tiles so the Tile
    # dependency tracker doesn't serialize consumers on unrelated writers) ---
    x_f32 = [big.tile([P, 512], F32, tag=f"xf{i}", name=f"xf{i}") for i in range(NT)]
    x_bf = [big.tile([P, 4, 512], BF16, tag=f"xb{g}", name=f"xb{g}") for g in range(NT // 4)]
    # xT_p[(c, g)][hl, j, t] = x[g*512+t, (2c+j)*128+hl]  (fp8, DoubleRow pairs)
    xT_p = {
        (c, g): big.tile([P, 2, 512], FP8, tag=f"xt{c}_{g}", name=f"xt{c}_{g}")
        for c in range(HC // 2) for g in range(NT // 4)
    }
    # h-major fp8 linear output. fp8e4m3 for the score matmuls (DoubleRow =
    # 2 contraction rows per PE cell -> half the matmuls). Softmax stats stay
    # fp32 and come from the same fp8 scores, so the diagonal exponentiates
    # to exactly 1 and the fp32-residual context path keeps full accuracy.
    outT = big.tile([P, HC, N], FP8)
    wT = big.tile([P, HC, H], FP8)         # k-major fp8 W (lhsT for linear)

    def x_bf_chunk(jc):
        return x_bf[jc // 4][:, jc % 4, :]

    # --- load + cast x tiles; build xT via PE identity transposes.
    # W is loaded over the gpsimd DMA queue in parallel with the x tiles on
    # the sync queue; its PE transposes are emitted after the first x group
    # so PE has work as early as possible. ---
    w_f32 = big.tile([P, HC, H], F32)
    w_bf = big.tile([P, HC, H], BF16)

    def load_x_group(g):
        dma_eng = nc.sync if g % 2 == 0 else nc.gpsimd
        for u in range(4):
            i = g * 4 + u
            dma_eng.dma_start(x_f32[i][:], x[i * P:(i + 1) * P, :])
            nc.vector.tensor_copy(x_bf[g][:, u, :], x_f32[i][:])
        for hc in range(HC):
            st = ps_stage.tile([P, 512], F32)
            for u in range(4):
                nc.tensor.matmul(
                    st[:, u * P:(u + 1) * P],
                    x_bf[g][:, u, hc * P:(hc + 1) * P],
                    ident[:],
                    start=True, stop=True,
                )
            if (g + hc) % 2 == 0:
                nc.vector.tensor_copy(xT_p[(hc // 2, g)][:, hc % 2, :], st[:])
            else:
                nc.scalar.copy(xT_p[(hc // 2, g)][:, hc % 2, :], st[:])

    nc.gpsimd.dma_start(w_f32[:], w.rearrange("(c p) k -> p c k", p=P))
    load_x_group(0)
    nc.vector.tensor_copy(w_bf[:], w_f32[:])
    for kc in range(HC):
        st = ps_stage.tile([P, 512], F32)
        for c in range(HC):
            nc.tensor.matmul(
                st[:, c * P:(c + 1) * P],
                w_bf[:, c, kc * P:(kc + 1) * P],
                ident[:],
                start=True, stop=True,
            )
        nc.vector.tensor_copy(wT[:, kc, :], st[:])
    for g in range(1, NT // 4):
        load_x_group(g)

    # --- linear: outT[hb] = wT^T @ xT + b (fp8 DoubleRow) ---
    for nt in range(FT):
        for hb in range(HC):
            ps = ps_mm.tile([P, 512], F32, tag="mm")
            for c in range(HC // 2):
                nc.tensor.matmul(
                    ps[:],
                    wT[:, 2 * c:2 * c + 2, hb * P:(hb + 1) * P],
                    xT_p[(c, nt)][:],
                    start=(c == 0), stop=(c == HC // 2 - 1),
                    perf_mode=mybir.MatmulPerfMode.DoubleRow,
                )
            nc.scalar.activation(
                outT[:, hb, nt * 512:(nt + 1) * 512],
                ps[:],
                mybir.ActivationFunctionType.Identity,
                bias=b_sb[:, hb:hb + 1],
                scale=1.0,
            )

    # --- negated score diagonal, used as the exp bias: d[q] = ||out_q||^2.
    # Squares of the *fp8-rounded* outT are exact in bf16, and the ones-matmul
    # accumulates them in fp32, so d matches the score diagonal to fp32
    # rounding -> exp(s_qq - d_q) == 1 and the residual context path stays
    # essentially exact (no per-block row-max needed at all). ---
    ones = const.tile([P, 1], BF16)
    nc.vector.memset(ones[:], 1.0)
    sq = [big.tile([P, N], BF16, tag=f"sq{hc}", name=f"sq{hc}") for hc in range(HC)]
    for hc in range(HC):
        nc.vector.tensor_mul(sq[hc][:], outT[:, hc, :], outT[:, hc, :])
    d_ps = ps_mm.tile([P, 512], F32, tag="mm")
    for q in range(NT):
        for hc in range(HC):
            nc.tensor.matmul(
                d_ps[:, q:q + 1],
                sq[hc][:, q * P:(q + 1) * P],
                ones[:],
                start=(hc == 0), stop=(hc == HC - 1),
            )
    negd = big.tile([P, NT], F32)
    nc.vector.tensor_scalar_mul(negd[:], d_ps[:, 0:NT], -1.0)

    # --- main attention loop: 1-deep software pipeline over query blocks ---
    ps_score = ctx.enter_context(tc.tile_pool(name="ps_score", bufs=2, space="PSUM"))

    def stage_a(q):
        """Scores + softmax numerator for block q; returns (pT3, sums, q)."""
        # exp + transpose pipelined per 1024-column half: with the fixed
        # diagonal bias there is no row-max pass, so each half's exp starts
        # the moment its matmuls finish.
        sums4 = stats.tile([P, 2], F32)
        pt3 = pt_pool.tile([P, NT, P], BF16)
        for h2 in range(2):
            sb = ps_score.tile([P, 1024], F32, tag="sc")
            for sub in range(2):
                jt = h2 * 2 + sub
                for c in range(HC // 2):
                    nc.tensor.matmul(
                        sb[:, sub * 512:(sub + 1) * 512],
                        outT[:, 2 * c:2 * c + 2, q * P:(q + 1) * P],
                        outT[:, 2 * c:2 * c + 2, jt * 512:(jt + 1) * 512],
                        start=(c == 0), stop=(c == HC // 2 - 1),
                        perf_mode=mybir.MatmulPerfMode.DoubleRow,
                    )
            p_j = p_pool.tile([P, 1024], BF16, tag=f"p{h2}", name=f"p{h2}")
            nc.scalar.activation(
                p_j[:], sb[:],
                mybir.ActivationFunctionType.Exp,
                bias=negd[:, q:q + 1], scale=1.0,
            )
            nc.sync.dma_start(
                pt3[:, 8 * h2:8 * (h2 + 1), :], p_j[:], transpose=True
            )
            # row-sums of the bf16-rounded p (consistent with what the
            # context matmul consumes, so the normalization is exact)
            nc.vector.tensor_reduce(
                sums4[:, h2:h2 + 1], p_j[:],
                axis=mybir.AxisListType.X, op=mybir.AluOpType.add,
            )
        sums = stats.tile([P, 1], F32)
        nc.vector.tensor_reduce(
            sums[:], sums4[:], axis=mybir.AxisListType.X, op=mybir.AluOpType.add,
        )
        # residual trick: subtract I on the (transposed) diagonal chunk
        nc.vector.tensor_sub(pt3[:, q, :], pt3[:, q, :], ident[:])
        return pt3, sums, q

    def stage_b(pt3, sums, q):
        """Context + normalize + store for block q."""
        ps_c = ps_mm.tile([P, 512], F32, tag="mm")
        for jc in range(NT):
            nc.tensor.matmul(
                ps_c[:],
                pt3[:, jc, :],
                x_bf_chunk(jc),
                start=(jc == 0), stop=(jc == NT - 1),
            )
        rinv = stats.tile([P, 1], F32)
        nc.vector.reciprocal(rinv[:], sums[:])
        ctx_sb = ctx_pool.tile([P, 512], F32)
        nc.vector.tensor_add(ctx_sb[:], ps_c[:], x_f32[q][:])
        nc.vector.tensor_scalar_mul(ctx_sb[:], ctx_sb[:], rinv[:])
        nc.sync.dma_start(out[q * P:(q + 1) * P, :], ctx_sb[:])

    # 2-deep pipeline: ctx for block q runs two score-blocks later, so PE
    # never waits on the exp/transpose chain.
    from collections import deque

    pending = deque()
    for q in range(NT):
        pending.append(stage_a(q))
        if len(pending) > 2:
            stage_b(*pending.popleft())
    while pending:
        stage_b(*pending.popleft())


def _get_nc():
    global _NC_CACHE
    if _NC_CACHE is None:
        from contextlib import ExitStack

        nc = bacc.Bacc(trn_type="TRN2", debug=False, num_devices=B)
        with tile.TileContext(nc) as tc:
            with ExitStack() as ctx:
                _build(ctx, tc)
        nc.compile()
        _NC_CACHE = nc
    return _NC_CACHE


def kernel(lstm_out: np.ndarray, W: np.ndarray, b: np.ndarray) -> np.ndarray:
    lstm_out = np.ascontiguousarray(lstm_out, dtype=np.float32)
    W = np.ascontiguousarray(W, dtype=np.float32)
    b = np.ascontiguousarray(b, dtype=np.float32)
    assert lstm_out.shape == (B, N, H), lstm_out.shape

    nc = _get_nc()
    in_maps = [
        {"x": lstm_out[i], "w": W, "bvec": b} for i in range(B)
    ]
    res = run_bass_kernel_spmd(nc, in_maps, core_ids=list(range(B)))
    return np.stack([r["out"] for r in res.results], axis=0)


if __name__ == "__main__":
    rng = np.random.default_rng(0)
    xs = rng.standard_normal((B, N, H), dtype=np.float32)
    Wm = rng.standard_normal((H, H), dtype=np.float32) * (1.0 / np.sqrt(H))
    bm = rng.standard_normal(H, dtype=np.float32) * (1.0 / np.sqrt(H))
    got = kernel(xs, Wm, bm)
    print("kernel output", got.shape, got.dtype)


# revision 40
# speedup vs baseline: 1.0187x; 1.0187x over previous
"""Trainium2 Bass kernel for nn_Attention (B=8, N=2048, H=512).

Reference computation (per batch b):
    out   = lstm_out @ W^T + b          # [N, H]
    score = out @ out^T                 # [N, N]
    attn  = softmax(score, axis=-1)
    ctx   = attn @ lstm_out             # [N, H]

Sharding: data-parallel over batch B across the 8 NeuronCores (one batch
element per core); W/b replicated. Each core runs an identical single-core
NEFF (SPMD, no collectives).

Per-core algorithm (matmuls in bf16 with fp32 PSUM accumulation; softmax
stats in fp32 with the exact row-max subtraction, like the reference):
  1. x [2048, 512] f32 -> SBUF, cast to bf16 (x_bf, token-major).
  2. xT (h-major) via xbar DMA transpose (4 chunked SBUF->SBUF transposes).
  3. outT[h, n] = W @ xT + b (h-major); lhsT = W^T built via PE
     identity-matmul transposes (W is tiny).
  4. per 128-query block (1-deep software pipeline so PE never stalls):
     stage A: S = outT^T @ outT -> PSUM [128, 2048] fp32 (16 matmuls),
       with per-bank partial row-max on DVE trailing the matmuls;
       p = exp(S - m) -> bf16 (ACT, fused per-partition bias and
       accumulated row-sums); subtract I from p's diagonal block;
       pT via one xbar DMA transpose -> [128, 16, 128].
     stage B (previous block): ctx = pT^T @ x_bf + x_f32, scaled by
       1/rowsum ("residual" form keeps the dominant diagonal term in
       exact fp32: attn@x = ((p - I) @ x + x) / rowsum(p)); DMA out.
"""

import sys

sys.path.insert(0, "/opt/trn_rl_repo")

import numpy as np

import concourse.bass as bass
import concourse.tile as tile
from concourse import bacc, mybir
from concourse.bass_utils import run_bass_kernel_spmd
from concourse.masks import make_identity

B, N, H = 8, 2048, 512
P = 128          # partitions
NT = N // P      # 16 token tiles
HC = H // P      # 4 h-chunks
FT = N // 512    # 4 free-dim tiles of 512 over tokens

F32 = mybir.dt.float32
BF16 = mybir.dt.bfloat16
FP8 = mybir.dt.float8e4

_NC_CACHE = None


def _build(ctx, tc):
    nc = tc.nc
    x = nc.dram_tensor("x", [N, H], F32, kind="ExternalInput").ap()
    w = nc.dram_tensor("w", [H, H], F32, kind="ExternalInput").ap()
    bvec = nc.dram_tensor("bvec", [H], F32, kind="ExternalInput").ap()
    out = nc.dram_tensor("out", [N, H], F32, kind="ExternalOutput").ap()

    const = ctx.enter_context(tc.tile_pool(name="const", bufs=1))
    big = ctx.enter_context(tc.tile_pool(name="big", bufs=1))
    p_pool = ctx.enter_context(tc.tile_pool(name="p", bufs=3))
    pt_pool = ctx.enter_context(tc.tile_pool(name="pt", bufs=3))
    stats = ctx.enter_context(tc.tile_pool(name="stats", bufs=12))
    ctx_pool = ctx.enter_context(tc.tile_pool(name="ctxp", bufs=3))

    ps_mm = ctx.enter_context(tc.tile_pool(name="ps_mm", bufs=2, space="PSUM"))
    ps_stage = ctx.enter_context(tc.tile_pool(name="ps_stage", bufs=2, space="PSUM"))

    # --- constants ---
    ident = const.tile([P, P], BF16)
    make_identity(nc, ident[:])
    b_sb = const.tile([P, HC], F32)
    nc.gpsimd.dma_start(b_sb[:], bvec.rearrange("(c p) -> p c", p=P))
    # (W also goes over the gpsimd DMA queue so it doesn't serialize behind
    # the x-tile loads on the sync queue)

    # --- persistent big tensors (split into per-group tiles so the Tile
    # dependency tracker doesn't serialize consumers on unrelated writers) ---
    x_f32 = [big.tile([P, 512], F32, tag=f"xf{i}", name=f"xf{i}") for i in range(NT)]
    x_bf = [big.tile([P, 4, 512], BF16, tag=f"xb{g}", name=f"xb{g}") for g in range(NT // 4)]
    # xT_p[(c, g)][hl, j, t] = x[g*512+t, (2c+j)*128+hl]  (fp8, DoubleRow pairs)
    xT_p = {
        (c, g): big.tile([P, 2, 512], FP8, tag=f"xt{c}_{g}", name=f"xt{c}_{g}")
        for c in range(HC // 2) for g in range(NT // 4)
    }
    # h-major fp8 linear output. fp8e4m3 for the score matmuls (DoubleRow =
    # 2 contraction rows per PE cell -> half the matmuls). Softmax stats stay
    # fp32 and come from the same fp8 scores, so the diagonal exponentiates
    # to exactly 1 and the fp32-residual context path keeps full accuracy.
    outT = big.tile([P, HC, N], FP8)
    wT = big.tile([P, HC, H], FP8)         # k-major fp8 W (lhsT for linear)

    def x_bf_chunk(jc):
        return x_bf[jc // 4][:, jc % 4, :]

    # --- load + cast x tiles; build xT via PE identity transposes.
    # W is loaded over the gpsimd DMA queue in parallel with the x tiles on
    # the sync queue; its PE transposes are emitted after the first x group
    # so PE has work as early as possible. ---
    w_f32 = big.tile([P, HC, H], F32)
    w_bf = big.tile([P, HC, H], BF16)

    def load_x_group(g):
        dma_eng = nc.sync if g % 2 == 0 else nc.gpsimd
        for u in range(4):
            i = g * 4 + u
            dma_eng.dma_start(x_f32[i][:], x[i * P:(i + 1) * P, :])
            nc.vector.tensor_copy(x_bf[g][:, u, :], x_f32[i][:])
        for hc in range(HC):
            st = ps_stage.tile([P, 512], F32)
            for u in range(4):
                nc.tensor.matmul(
                    st[:, u * P:(u + 1) * P],
                    x_bf[g][:, u, hc * P:(hc + 1) * P],
                    ident[:],
                    start=True, stop=True,
                )
            if (g + hc) % 2 == 0:
                nc.vector.tensor_copy(xT_p[(hc // 2, g)][:, hc % 2, :], st[:])
            else:
                nc.scalar.copy(xT_p[(hc // 2, g)][:, hc % 2, :], st[:])

    nc.gpsimd.dma_start(w_f32[:], w.rearrange("(c p) k -> p c k", p=P))
    load_x_group(0)
    nc.vector.tensor_copy(w_bf[:], w_f32[:])
    for kc in range(HC):
        st = ps_stage.tile([P, 512], F32)
        for c in range(HC):
            nc.tensor.matmul(
                st[:, c * P:(c + 1) * P],
                w_bf[:, c, kc * P:(kc + 1) * P],
                ident[:],
                start=True, stop=True,
            )
        nc.vector.tensor_copy(wT[:, kc, :], st[:])
    for g in range(1, NT // 4):
        load_x_group(g)

    # --- linear: outT[hb] = wT^T @ xT + b (fp8 DoubleRow) ---
    for nt in range(FT):
        for hb in range(HC):
            ps = ps_mm.tile([P, 512], F32, tag="mm")
            for c in range(HC // 2):
                nc.tensor.matmul(
                    ps[:],
                    wT[:, 2 * c:2 * c + 2, hb * P:(hb + 1) * P],
                    xT_p[(c, nt)][:],
                    start=(c == 0), stop=(c == HC // 2 - 1),
                    perf_mode=mybir.MatmulPerfMode.DoubleRow,
                )
            nc.scalar.activation(
                outT[:, hb, nt * 512:(nt + 1) * 512],
                ps[:],
                mybir.ActivationFunctionType.Identity,
                bias=b_sb[:, hb:hb + 1],
                scale=1.0,
            )

    # --- negated score diagonal, used as the exp bias: d[q] = ||out_q||^2.
    # Squares of the *fp8-rounded* outT are exact in bf16, and the ones-matmul
    # accumulates them in fp32, so d matches the score diagonal to fp32
    # rounding -> exp(s_qq - d_q) ~= 1 and the residual context path stays
    # essentially exact (no per-block row-max needed at all).
    # Emitted lazily (after block 0's score matmuls) so the DVE squares
    # overlap PE score work instead of stalling it. ---
    ones = const.tile([P, 1], BF16)
    nc.vector.memset(ones[:], 1.0)
    negd = big.tile([P, NT], F32)

    def emit_d_phase():
        sq = [
            big.tile([P, N], BF16, tag=f"sq{hc}", name=f"sq{hc}")
            for hc in range(HC)
        ]
        for hc in range(HC):
            nc.vector.tensor_mul(sq[hc][:], outT[:, hc, :], outT[:, hc, :])
        d_ps = ps_mm.tile([P, 512], F32, tag="mm")
        for q in range(NT):
            for hc in range(HC):
                nc.tensor.matmul(
                    d_ps[:, q:q + 1],
                    sq[hc][:, q * P:(q + 1) * P],
                    ones[:],
                    start=(hc == 0), stop=(hc == HC - 1),
                )
        nc.vector.tensor_scalar_mul(negd[:], d_ps[:, 0:NT], -1.0)

    # --- main attention loop: 1-deep software pipeline over query blocks ---
    ps_score = ctx.enter_context(tc.tile_pool(name="ps_score", bufs=2, space="PSUM"))

    def score_half(q, h2):
        sb = ps_score.tile([P, 1024], F32, tag="sc", name="sb")
        for sub in range(2):
            jt = h2 * 2 + sub
            for c in range(HC // 2):
                nc.tensor.matmul(
                    sb[:, sub * 512:(sub + 1) * 512],
                    outT[:, 2 * c:2 * c + 2, q * P:(q + 1) * P],
                    outT[:, 2 * c:2 * c + 2, jt * 512:(jt + 1) * 512],
                    start=(c == 0), stop=(c == HC // 2 - 1),
                    perf_mode=mybir.MatmulPerfMode.DoubleRow,
                )
        return sb

    def softmax_half(q, h2, sb, pt3, sums4):
        # exp + transpose pipelined per 1024-column half: with the fixed
        # diagonal bias there is no row-max pass, so each half's exp starts
        # the moment its matmuls finish.
        p_j = p_pool.tile([P, 1024], BF16, tag=f"p{h2}", name=f"p{h2}")
        nc.scalar.activation(
            p_j[:], sb[:],
            mybir.ActivationFunctionType.Exp,
            bias=negd[:, q:q + 1], scale=1.0,
        )
        nc.sync.dma_start(
            pt3[:, 8 * h2:8 * (h2 + 1), :], p_j[:], transpose=True
        )
        # row-sums of the bf16-rounded p (consistent with what the
        # context matmul consumes, so the normalization is exact)
        nc.vector.tensor_reduce(
            sums4[:, h2:h2 + 1], p_j[:],
            axis=mybir.AxisListType.X, op=mybir.AluOpType.add,
        )

    def stage_a(q, mid_hook=None):
        """Scores + softmax numerator for block q; returns (pT3, sums, q)."""
        sums4 = stats.tile([P, 2], F32)
        pt3 = pt_pool.tile([P, NT, P], BF16)
        halves = [score_half(q, h2) for h2 in range(2)]
        if mid_hook is not None:
            mid_hook()
        for h2 in range(2):
            softmax_half(q, h2, halves[h2], pt3, sums4)
        sums = stats.tile([P, 1], F32)
        nc.vector.tensor_reduce(
            sums[:], sums4[:], axis=mybir.AxisListType.X, op=mybir.AluOpType.add,
        )
        # residual trick: subtract I on the (transposed) diagonal chunk
        nc.vector.tensor_sub(pt3[:, q, :], pt3[:, q, :], ident[:])
        return pt3, sums, q

    def stage_b(pt3, sums, q):
        """Context + normalize + store for block q."""
        ps_c = ps_mm.tile([P, 512], F32, tag="mm")
        for jc in range(NT):
            nc.tensor.matmul(
                ps_c[:],
                pt3[:, jc, :],
                x_bf_chunk(jc),
                start=(jc == 0), stop=(jc == NT - 1),
            )
        rinv = stats.tile([P, 1], F32)
        nc.vector.reciprocal(rinv[:], sums[:])
        ctx_sb = ctx_pool.tile([P, 512], F32)
        nc.vector.tensor_add(ctx_sb[:], ps_c[:], x_f32[q][:])
        nc.vector.tensor_scalar_mul(ctx_sb[:], ctx_sb[:], rinv[:])
        nc.sync.dma_start(out[q * P:(q + 1) * P, :], ctx_sb[:])

    # 2-deep pipeline: ctx for block q runs two score-blocks later, so PE
    # never waits on the exp/transpose chain. The d-phase matmuls slot in
    # right after block 0's score matmuls (block 0's exp waits on negd).
    from collections import deque

    pending = deque()
    for q in range(NT):
        pending.append(stage_a(q, mid_hook=emit_d_phase if q == 0 else None))
        if len(pending) > 2:
            stage_b(*pending.popleft())
    while pending:
        stage_b(*pending.popleft())


def _get_nc():
    global _NC_CACHE
    if _NC_CACHE is None:
        from contextlib import ExitStack

        nc = bacc.Bacc(trn_type="TRN2", debug=False, num_devices=B)
        with tile.TileContext(nc) as tc:
            with ExitStack() as ctx:
                _build(ctx, tc)
        nc.compile()
        _NC_CACHE = nc
    return _NC_CACHE


def kernel(lstm_out: np.ndarray, W: np.ndarray, b: np.ndarray) -> np.ndarray:
    lstm_out = np.ascontiguousarray(lstm_out, dtype=np.float32)
    W = np.ascontiguousarray(W, dtype=np.float32)
    b = np.ascontiguousarray(b, dtype=np.float32)
    assert lstm_out.shape == (B, N, H), lstm_out.shape

    nc = _get_nc()
    in_maps = [
        {"x": lstm_out[i], "w": W, "bvec": b} for i in range(B)
    ]
    res = run_bass_kernel_spmd(nc, in_maps, core_ids=list(range(B)))
    return np.stack([r["out"] for r in res.results], axis=0)


if __name__ == "__main__":
    rng = np.random.default_rng(0)
    xs = rng.standard_normal((B, N, H), dtype=np.float32)
    Wm = rng.standard_normal((H, H), dtype=np.float32) * (1.0 / np.sqrt(H))
    bm = rng.standard_normal(H, dtype=np.float32) * (1.0 / np.sqrt(H))
    got = kernel(xs, Wm, bm)
    print("kernel output", got.shape, got.dtype)
